# revision 24
# baseline (speedup 1.0000x reference)
"""Single-head attention (B=4, S=2048, E=1024, fp32) on 8 trn2 NeuronCores.

Sharding: (batch, q-half) -> 8 shards. Core c handles batch c//2, query/seq
rows [h*1024, (h+1)*1024) with h = c%2. Unlike the fp32r baseline, each core
computes K and V projections ONLY for its own 1024-row half; the partner's
half arrives via a pairwise AllGather (DRAM->DRAM). Because the program is
SPMD (identical on both cores of a pair), the partner slice index of the
gather output is rank-dependent; instead of rank addressing, the partner
half is reconstructed exactly on the vector engine as
    partner = (gather[0] + gather[1]) - own
computed in fp32 (bf16 + bf16 is exact in fp32), written back as bf16.

All matmul operands are bf16 (hw streams 2 rows/cycle vs 1 for fp32r; PSUM
accumulation is fp32). Matmul groups are emitted "A-pattern": the 8-16
matmuls of one accumulation group are consecutive into a single PSUM bank
(bank alternation costs ~60ns/matmul on hw).

Per-core k-order is own-half-first (softmax/output invariant to key order
as long as PT and V use the same order).

Device layouts (per core, partition dim first):
  xt    [P, 8(et), 1024]  x[b].T own columns       (bf16)
  qt    [P, 8(ft), 1024]  Q^T                      (bf16)
  kt    [P, 8(ft), 2048]  K^T own|partner          (bf16)
  v     [P, 16(st), 1024] V own|partner            (bf16)
  pt    [P, 16(kt), 1024] exp(scores^T)            (bf16)
"""

import numpy as np

P = 128
E = 1024
S = 2048
SQ = 1024          # q rows per core
ET = E // P        # 8 e/f tiles
QTN = SQ // P      # 8 q tiles
STH = SQ // P      # 8 k tiles per half
ST = S // P        # 16 k tiles


def _emit(nc):
    import concourse.mybir as mybir
    import concourse.tile as tile

    f32 = mybir.dt.float32
    bf16 = mybir.dt.bfloat16
    ACT = mybir.ActivationFunctionType

    xt_in = nc.dram_tensor("xt_in", [2, P, ET, 512], bf16, kind="ExternalInput")
    wkf0_in = nc.dram_tensor("wkf0_in", [P, ET, P], bf16, kind="ExternalInput")
    wq4 = nc.dram_tensor("wq4", [P, ET, ET, P], bf16, kind="ExternalInput")
    wk4 = nc.dram_tensor("wk4", [P, ET, ET, P], bf16, kind="ExternalInput")
    wvt_in = nc.dram_tensor("wvt_in", [P, ET, E], bf16, kind="ExternalInput")
    bq8 = nc.dram_tensor("bq8", [P, ET], f32, kind="ExternalInput")
    bk8 = nc.dram_tensor("bk8", [P, ET], f32, kind="ExternalInput")
    bvb = nc.dram_tensor("bvb", [P, E], f32, kind="ExternalInput")
    ones2 = nc.dram_tensor("ones2", [P, 2], bf16, kind="ExternalInput")
    id2 = nc.dram_tensor("id2", [2, 2], f32, kind="ExternalInput")
    o = nc.dram_tensor("o", [SQ, E], f32, kind="ExternalOutput")

    RG = [[0, 1], [2, 3], [4, 5], [6, 7]]

    with tile.TileContext(nc) as tc:
        dram_cm = tc.tile_pool(name="dramp", bufs=1, space="DRAM")
        dramp = dram_cm.__enter__()
        kcc_src = dramp.tile([P, ET, SQ], bf16, tag="kccs")    # [p, ft, k_own]
        kcc_dst = dramp.tile([2, P, ET, SQ], bf16, tag="kccd")
        vcc_src = dramp.tile([P, STH, E], bf16, tag="vccs")    # [p, st, f]
        vcc_dst = dramp.tile([2, P, STH, E], bf16, tag="vccd")

        psum_cm = tc.tile_pool(name="psum", bufs=4, space="PSUM")
        psum = psum_cm.__enter__()
        small_cm = tc.tile_pool(name="small", bufs=1)
        small = small_cm.__enter__()

        # long-lived big tiles
        big_cm = tc.tile_pool(name="big", bufs=1)
        big = big_cm.__enter__()
        kt_t = big.tile([P, 2, ET, SQ], bf16, tag="kt")
        v_t = big.tile([P, ST, E], bf16, tag="v")

        bq_t = small.tile([P, ET], f32, tag="bq")
        nc.gpsimd.dma_start(bq_t[:], bq8[:])
        bk_t = small.tile([P, ET], f32, tag="bk")
        nc.gpsimd.dma_start(bk_t[:], bk8[:])
        bv_t = small.tile([P, E], f32, tag="bv")
        nc.gpsimd.dma_start(bv_t[:], bvb[:])
        ones_t = small.tile([P, 2], bf16, tag="ones")
        nc.gpsimd.dma_start(ones_t[:], ones2[:])
        id2_t = small.tile([2, 2], f32, tag="id2")
        nc.gpsimd.dma_start(id2_t[:], id2[:])
        rs_sb = small.tile([2, SQ], f32, tag="rssb")

        # ---- phase 1: K (own half) -> kick CC; V (own half) -> kick CC; Q ----
        mid_cm = tc.tile_pool(name="mid", bufs=1)
        mid = mid_cm.__enter__()
        xt_c = [
            mid.tile([P, ET, 512], bf16, tag=f"xt{c}", name=f"xt{c}")
            for c in range(2)
        ]
        qt_t = mid.tile([P, ET, SQ], bf16, tag="qt")
        gat_cm = tc.tile_pool(name="gat", bufs=1)
        gat = gat_cm.__enter__()
        g0 = gat.tile([P, ET // 2, SQ], bf16, tag="g0")
        g1 = gat.tile([P, ET // 2, SQ], bf16, tag="g1")
        gsum = gat.tile([P, 512], f32, tag="gsum")
        w_cm = tc.tile_pool(name="wstream", bufs=1)
        wsp = w_cm.__enter__()
        wvt_t = wsp.tile([P, ET, E], bf16, tag="wvt")

        # input DMAs: partition-major DRAM layouts, single contiguous
        # transfers, split across the two HWDGE queues for parallelism
        wk_f0 = wsp.tile([P, ET, P], bf16, tag="wf0", name="wk_f0")
        nc.sync.dma_start(wk_f0[:], wkf0_in[:])
        nc.sync.dma_start(xt_c[0][:], xt_in[0])
        nc.sync.dma_start(xt_c[1][:], xt_in[1])
        wk_all = wsp.tile([P, ET, ET, P], bf16, tag="wk", name="wk_all")
        nc.scalar.dma_start(wk_all[:], wk4[:])
        wq_all = wsp.tile([P, ET, ET, P], bf16, tag="wq", name="wq_all")
        nc.sync.dma_start(wq_all[:], wq4[:])
        nc.scalar.dma_start(wvt_t[:], wvt_in[:])

        def wk_row(f):
            return wk_f0 if f == 0 else wk_all[:, f]

        # K projection (own half): out kt[:, f, 0:1024]
        for f in range(ET):
            for j in range(2):
                pst = psum.tile([P, 512], f32, tag="mm", name=f"psk{j}")
                for e in range(ET):
                    nc.tensor.matmul(
                        pst[:],
                        wk_row(f)[:, e],
                        xt_c[j][:, e],
                        start=(e == 0),
                        stop=(e == ET - 1),
                    )
                nc.scalar.add(
                    kt_t[:, 0, f, j * 512 : (j + 1) * 512],
                    pst[:],
                    bk_t[:, f : f + 1],
                )
        # one contiguous store of the own K^T half for the collective
        nc.sync.dma_start(kcc_src[:], kt_t[:, 0])
        nc.gpsimd.collective_compute(
            "AllGather",
            mybir.AluOpType.bypass,
            replica_groups=RG,
            ins=[kcc_src[:]],
            outs=[kcc_dst[:]],
        )
        # partner-K gather round 1: first 4 f-tiles of each pair member
        nc.sync.dma_start(g0[:], kcc_dst[0, :, 0 : ET // 2])
        nc.sync.dma_start(g1[:], kcc_dst[1, :, 0 : ET // 2])

        # V projection (own half): out v[:, st, :] for st in 0..8
        for st in range(STH):
            for fc in range(2):
                pst = psum.tile([P, 512], f32, tag="mm", name=f"psv{fc}")
                for e in range(ET):
                    nc.tensor.matmul(
                        pst[:],
                        xt_c[st // 4][:, e, (st % 4) * P : (st % 4 + 1) * P],
                        wvt_t[:, e, fc * 512 : (fc + 1) * 512],
                        start=(e == 0),
                        stop=(e == ET - 1),
                    )
                nc.vector.tensor_add(
                    v_t[:, st, fc * 512 : (fc + 1) * 512],
                    pst[:],
                    bv_t[:, fc * 512 : (fc + 1) * 512],
                )
        # Q projection: out qt
        for j in range(2):
            for f in range(ET):
                pst = psum.tile([P, 512], f32, tag="mm", name=f"psq{f%2}")
                for e in range(ET):
                    nc.tensor.matmul(
                        pst[:],
                        wq_all[:, f, e],
                        xt_c[j][:, e],
                        start=(e == 0),
                        stop=(e == ET - 1),
                    )
                nc.scalar.add(
                    qt_t[:, f, j * 512 : (j + 1) * 512], pst[:], bq_t[:, f : f + 1]
                )
        w_cm.__exit__(None, None, None)

        # ---- partner reconstruction: kt partner half, 2 rounds ----
        for rnd in range(2):
            if rnd == 1:
                nc.sync.dma_start(g0[:], kcc_dst[0, :, ET // 2 : ET])
                nc.sync.dma_start(g1[:], kcc_dst[1, :, ET // 2 : ET])
            for fh in range(ET // 2):
                f = rnd * (ET // 2) + fh
                for c in range(2):
                    sl = slice(c * 512, (c + 1) * 512)
                    nc.vector.tensor_add(gsum[:], g0[:, fh, sl], g1[:, fh, sl])
                    nc.vector.tensor_sub(
                        kt_t[:, 1, f, sl], gsum[:], kt_t[:, 0, f, sl]
                    )

        # V exchange: store + collective fire after the K gathers are done
        # (dummy byte write pins the queue/scheduler order)
        nc.sync.dma_start(vcc_src[0:1, 0, 0:2], g1[0:1, 0, 0:2])
        nc.sync.dma_start(vcc_src[:], v_t[:, 0:STH, :])
        nc.gpsimd.collective_compute(
            "AllGather",
            mybir.AluOpType.bypass,
            replica_groups=RG,
            ins=[vcc_src[:]],
            outs=[vcc_dst[:]],
        )

        # pt lives in the space freed by the weight + kg pools (avoids WAR
        # coupling of exp writes to unrelated late readers)
        pt_cm = tc.tile_pool(name="ptp", bufs=1)
        ptp = pt_cm.__enter__()
        pt_t = ptp.tile([P, ST, SQ], bf16, tag="pt")

        # ---- partner reconstruction: v[:, 8:16, :] (overlaps scores) ----
        for rnd in range(2):
            nc.sync.dma_start(g0[:], vcc_dst[0, :, rnd * (STH // 2) : (rnd + 1) * (STH // 2)])
            nc.sync.dma_start(g1[:], vcc_dst[1, :, rnd * (STH // 2) : (rnd + 1) * (STH // 2)])
            for sh in range(STH // 2):
                st = rnd * (STH // 2) + sh
                for c in range(2):
                    sl = slice(c * 512, (c + 1) * 512)
                    nc.vector.tensor_add(gsum[:], g0[:, sh, sl], g1[:, sh, sl])
                    nc.vector.tensor_sub(
                        v_t[:, STH + st, sl], gsum[:], v_t[:, st, sl]
                    )

        # ---- phase 2: scores^T + exp -> pt ----
        for k in range(ST):
            for qc in range(2):
                ps = psum.tile([P, 512], f32, tag="mm", name="pss")
                for f in range(ET):
                    nc.tensor.matmul(
                        ps[:],
                        kt_t[:, k // STH, f, (k % STH) * P : (k % STH + 1) * P],
                        qt_t[:, f, qc * 512 : (qc + 1) * 512],
                        start=(f == 0),
                        stop=(f == ET - 1),
                    )
                nc.scalar.activation(
                    pt_t[:, k, qc * 512 : (qc + 1) * 512], ps[:], ACT.Exp
                )

        # rowsums batched after scores: clean same-bank matmul streaks
        rs_cm = tc.tile_pool(name="rsp", bufs=1, space="PSUM")
        rsp = rs_cm.__enter__()
        rs_ps = [
            rsp.tile([2, 512], f32, tag=f"rsacc{qc}", name=f"rsacc{qc}")
            for qc in range(2)
        ]
        for qc in range(2):
            for k in range(ST):
                nc.tensor.matmul(
                    rs_ps[qc][:],
                    ones_t[:],
                    pt_t[:, k, qc * 512 : (qc + 1) * 512],
                    start=(k == 0),
                    stop=(k == ST - 1),
                )
        for qc in range(2):
            nc.vector.tensor_copy(rs_sb[:, qc * 512 : (qc + 1) * 512], rs_ps[qc][:])
        rs_cm.__exit__(None, None, None)

        # ---- phase 4: O = PT^T @ V, normalized ----
        prs_cm = tc.tile_pool(name="prsp", bufs=2, space="PSUM")
        prsp = prs_cm.__enter__()
        ob_cm = tc.tile_pool(name="ob", bufs=3)
        obp = ob_cm.__enter__()
        recip_t = small.tile([P, QTN], f32, tag="recip")
        for qt_i in range(QTN):
            prs = prsp.tile([P, 2], f32, tag="rs")
            nc.tensor.matmul(
                prs[:],
                rs_sb[:, qt_i * P : (qt_i + 1) * P],
                id2_t[:],
                is_transpose=True,
            )
            nc.vector.reciprocal(recip_t[:, qt_i : qt_i + 1], prs[:, 0:1])
        for qt_i in range(QTN):
            for fc in range(2):
                po = psum.tile([P, 512], f32, tag="mm", name="po")
                for k in range(ST):
                    nc.tensor.matmul(
                        po[:],
                        pt_t[:, k, qt_i * P : (qt_i + 1) * P],
                        v_t[:, k, fc * 512 : (fc + 1) * 512],
                        start=(k == 0),
                        stop=(k == ST - 1),
                    )
                o_t = obp.tile([P, 512], f32, tag="ob")
                nc.vector.tensor_scalar_mul(
                    o_t[:], po[:], recip_t[:, qt_i : qt_i + 1]
                )
                nc.scalar.dma_start(
                    o[qt_i * P : (qt_i + 1) * P, fc * 512 : (fc + 1) * 512],
                    o_t[:],
                )
        ob_cm.__exit__(None, None, None)
        prs_cm.__exit__(None, None, None)

        pt_cm.__exit__(None, None, None)
        w2 = None  # placeholder
        gat_cm.__exit__(None, None, None)
        mid_cm.__exit__(None, None, None)
        big_cm.__exit__(None, None, None)
        small_cm.__exit__(None, None, None)
        psum_cm.__exit__(None, None, None)
        dram_cm.__exit__(None, None, None)


_NC_CACHE = {}


def build_nc():
    if "nc" in _NC_CACHE:
        return _NC_CACHE["nc"]
    import concourse.bacc as bacc

    nc = bacc.Bacc(None, target_bir_lowering=False, num_devices=8)
    _emit(nc)
    nc.finalize()
    _NC_CACHE["nc"] = nc
    return nc


def make_in_maps(x, Wq, bq, Wk, bk, Wv, bv):
    """Host-side prep: per-core input dicts for run_bass_kernel_spmd."""
    import ml_dtypes

    bf16 = ml_dtypes.bfloat16
    scale = 1.0 / np.sqrt(np.float32(E))
    x = np.asarray(x, np.float32)
    B = x.shape[0]

    def tile4(wt):  # [E, E] (e rows, f cols) -> [p(e), f_tile, e_tile, c(f)]
        return np.ascontiguousarray(
            wt.reshape(ET, P, ET, P).transpose(1, 2, 0, 3)
        )

    wq4 = tile4(np.asarray(Wq, np.float32).T * scale).astype(bf16)
    wk4 = tile4(np.asarray(Wk, np.float32).T).astype(bf16)
    wkf0 = np.ascontiguousarray(wk4[:, 0])
    wvt = np.ascontiguousarray(
        np.asarray(Wv, np.float32).T.reshape(ET, P, E).transpose(1, 0, 2)
    ).astype(bf16)
    bq8 = np.ascontiguousarray((np.asarray(bq, np.float32) * scale).reshape(ET, P).T)
    bk8 = np.ascontiguousarray(np.asarray(bk, np.float32).reshape(ET, P).T)
    bvb = np.ascontiguousarray(np.broadcast_to(np.asarray(bv, np.float32), (P, E)))

    in_maps = []
    for c in range(B * 2):
        b, h = divmod(c, 2)
        xt_pm = (
            x[b].T[:, h * SQ : (h + 1) * SQ]
            .reshape(ET, P, 2, 512)
            .transpose(2, 1, 0, 3)
        )
        xt_half = np.ascontiguousarray(xt_pm).astype(bf16)
        in_maps.append(
            {
                "xt_in": xt_half,
                "wkf0_in": wkf0,
                "wq4": wq4,
                "wk4": wk4,
                "wvt_in": wvt,
                "bq8": bq8,
                "bk8": bk8,
                "bvb": bvb,
                "ones2": np.ones((P, 2), bf16),
                "id2": np.eye(2, dtype=np.float32),
            }
        )
    return in_maps


def kernel(x, Wq, bq, Wk, bk, Wv, bv):
    from concourse.bass_utils import run_bass_kernel_spmd

    x = np.asarray(x, np.float32)
    B = x.shape[0]
    nc = build_nc()
    in_maps = make_in_maps(x, Wq, bq, Wk, bk, Wv, bv)
    res = run_bass_kernel_spmd(nc, in_maps, list(range(len(in_maps))))
    out = np.empty((B, S, E), np.float32)
    for c in range(len(in_maps)):
        b, h = divmod(c, 2)
        out[b, h * SQ : (h + 1) * SQ, :] = res.results[c]["o"]
    return out


# revision 25
# speedup vs baseline: 1.0394x; 1.0394x over previous
"""Single-head attention (B=4, S=2048, E=1024, fp32) on 8 trn2 NeuronCores.

Sharding: (batch, q-half) -> 8 shards. Core c handles batch c//2, query/seq
rows [h*1024, (h+1)*1024) with h = c%2. Unlike the fp32r baseline, each core
computes K and V projections ONLY for its own 1024-row half; the partner's
half arrives via a pairwise AllGather (DRAM->DRAM). Because the program is
SPMD (identical on both cores of a pair), the partner slice index of the
gather output is rank-dependent; instead of rank addressing, the partner
half is reconstructed exactly on the vector engine as
    partner = (gather[0] + gather[1]) - own
computed in fp32 (bf16 + bf16 is exact in fp32), written back as bf16.

All matmul operands are bf16 (hw streams 2 rows/cycle vs 1 for fp32r; PSUM
accumulation is fp32). Matmul groups are emitted "A-pattern": the 8-16
matmuls of one accumulation group are consecutive into a single PSUM bank
(bank alternation costs ~60ns/matmul on hw).

Per-core k-order is own-half-first (softmax/output invariant to key order
as long as PT and V use the same order).

Device layouts (per core, partition dim first):
  xt    [P, 8(et), 1024]  x[b].T own columns       (bf16)
  qt    [P, 8(ft), 1024]  Q^T                      (bf16)
  kt    [P, 8(ft), 2048]  K^T own|partner          (bf16)
  v     [P, 16(st), 1024] V own|partner            (bf16)
  pt    [P, 16(kt), 1024] exp(scores^T)            (bf16)
"""

import numpy as np

P = 128
E = 1024
S = 2048
SQ = 1024          # q rows per core
ET = E // P        # 8 e/f tiles
QTN = SQ // P      # 8 q tiles
STH = SQ // P      # 8 k tiles per half
ST = S // P        # 16 k tiles


def _emit(nc):
    import concourse.mybir as mybir
    import concourse.tile as tile

    f32 = mybir.dt.float32
    bf16 = mybir.dt.bfloat16
    ACT = mybir.ActivationFunctionType

    xt_in = nc.dram_tensor("xt_in", [2, P, ET, 512], bf16, kind="ExternalInput")
    wkf0_in = nc.dram_tensor("wkf0_in", [P, ET, P], bf16, kind="ExternalInput")
    wq4 = nc.dram_tensor("wq4", [P, ET, ET, P], bf16, kind="ExternalInput")
    wk4 = nc.dram_tensor("wk4", [P, ET, ET, P], bf16, kind="ExternalInput")
    wvt_in = nc.dram_tensor("wvt_in", [P, ET, E], bf16, kind="ExternalInput")
    bq8 = nc.dram_tensor("bq8", [P, ET], f32, kind="ExternalInput")
    bk8 = nc.dram_tensor("bk8", [P, ET], f32, kind="ExternalInput")
    bvb = nc.dram_tensor("bvb", [P, E], f32, kind="ExternalInput")
    ones2 = nc.dram_tensor("ones2", [P, 2], bf16, kind="ExternalInput")
    id2 = nc.dram_tensor("id2", [2, 2], f32, kind="ExternalInput")
    o = nc.dram_tensor("o", [SQ, E], f32, kind="ExternalOutput")

    RG = [[0, 1], [2, 3], [4, 5], [6, 7]]

    with tile.TileContext(nc) as tc:
        dram_cm = tc.tile_pool(name="dramp", bufs=1, space="DRAM")
        dramp = dram_cm.__enter__()
        kcc_src = dramp.tile([P, ET, SQ], bf16, tag="kccs")    # [p, ft, k_own]
        kcc_dst = dramp.tile([2, P, ET, SQ], bf16, tag="kccd")
        vcc_src = dramp.tile([P, STH, E], bf16, tag="vccs")    # [p, st, f]
        vcc_dst = dramp.tile([2, P, STH, E], bf16, tag="vccd")

        psum_cm = tc.tile_pool(name="psum", bufs=4, space="PSUM")
        psum = psum_cm.__enter__()
        small_cm = tc.tile_pool(name="small", bufs=1)
        small = small_cm.__enter__()

        # long-lived big tiles
        big_cm = tc.tile_pool(name="big", bufs=1)
        big = big_cm.__enter__()
        kt_t = big.tile([P, 2, ET, SQ], bf16, tag="kt")
        v_t = big.tile([P, ST, E], bf16, tag="v")

        bq_t = small.tile([P, ET], f32, tag="bq")
        nc.gpsimd.dma_start(bq_t[:], bq8[:])
        bk_t = small.tile([P, ET], f32, tag="bk")
        nc.gpsimd.dma_start(bk_t[:], bk8[:])
        bv_t = small.tile([P, E], f32, tag="bv")
        nc.gpsimd.dma_start(bv_t[:], bvb[:])
        ones_t = small.tile([P, 2], bf16, tag="ones")
        nc.gpsimd.dma_start(ones_t[:], ones2[:])
        id2_t = small.tile([2, 2], f32, tag="id2")
        nc.gpsimd.dma_start(id2_t[:], id2[:])
        rs_sb = small.tile([2, SQ], f32, tag="rssb")

        # ---- phase 1: K (own half) -> kick CC; V (own half) -> kick CC; Q ----
        mid_cm = tc.tile_pool(name="mid", bufs=1)
        mid = mid_cm.__enter__()
        xt_c = [
            mid.tile([P, ET, 512], bf16, tag=f"xt{c}", name=f"xt{c}")
            for c in range(2)
        ]
        qt_t = mid.tile([P, ET, SQ], bf16, tag="qt")
        gat_cm = tc.tile_pool(name="gat", bufs=1)
        gat = gat_cm.__enter__()
        g0 = gat.tile([P, ET // 2, SQ], bf16, tag="g0")
        g1 = gat.tile([P, ET // 2, SQ], bf16, tag="g1")
        gsum = gat.tile([P, 512], f32, tag="gsum")
        w_cm = tc.tile_pool(name="wstream", bufs=1)
        wsp = w_cm.__enter__()
        wvt_t = wsp.tile([P, ET, E], bf16, tag="wvt")

        # input DMAs: partition-major DRAM layouts, single contiguous
        # transfers, split across the two HWDGE queues for parallelism
        wk_f0 = wsp.tile([P, ET, P], bf16, tag="wf0", name="wk_f0")
        nc.sync.dma_start(wk_f0[:], wkf0_in[:])
        nc.sync.dma_start(xt_c[0][:], xt_in[0])
        nc.sync.dma_start(xt_c[1][:], xt_in[1])
        wk_all = wsp.tile([P, ET, ET, P], bf16, tag="wk", name="wk_all")
        nc.scalar.dma_start(wk_all[:], wk4[:])
        wq_all = wsp.tile([P, ET, ET, P], bf16, tag="wq", name="wq_all")
        nc.sync.dma_start(wq_all[:], wq4[:])
        nc.scalar.dma_start(wvt_t[:], wvt_in[:])

        def wk_row(f):
            return wk_f0 if f == 0 else wk_all[:, f]

        # K projection (own half): out kt[:, f, 0:1024]
        for f in range(ET):
            for j in range(2):
                pst = psum.tile([P, 512], f32, tag="mm", name=f"psk{j}")
                for e in range(ET):
                    nc.tensor.matmul(
                        pst[:],
                        wk_row(f)[:, e],
                        xt_c[j][:, e],
                        start=(e == 0),
                        stop=(e == ET - 1),
                    )
                nc.scalar.add(
                    kt_t[:, 0, f, j * 512 : (j + 1) * 512],
                    pst[:],
                    bk_t[:, f : f + 1],
                )
        # one contiguous store of the own K^T half for the collective
        nc.sync.dma_start(kcc_src[:], kt_t[:, 0])
        nc.gpsimd.collective_compute(
            "AllGather",
            mybir.AluOpType.bypass,
            replica_groups=RG,
            ins=[kcc_src[:]],
            outs=[kcc_dst[:]],
        )
        # partner-K gather round 1: first 4 f-tiles of each pair member
        nc.sync.dma_start(g0[:], kcc_dst[0, :, 0 : ET // 2])
        nc.sync.dma_start(g1[:], kcc_dst[1, :, 0 : ET // 2])

        # V projection (own half): out v[:, st, :] for st in 0..8
        for st in range(STH):
            for fc in range(2):
                pst = psum.tile([P, 512], f32, tag="mm", name=f"psv{fc}")
                for e in range(ET):
                    nc.tensor.matmul(
                        pst[:],
                        xt_c[st // 4][:, e, (st % 4) * P : (st % 4 + 1) * P],
                        wvt_t[:, e, fc * 512 : (fc + 1) * 512],
                        start=(e == 0),
                        stop=(e == ET - 1),
                    )
                nc.vector.tensor_add(
                    v_t[:, st, fc * 512 : (fc + 1) * 512],
                    pst[:],
                    bv_t[:, fc * 512 : (fc + 1) * 512],
                )
        # Q projection: out qt
        for j in range(2):
            for f in range(ET):
                pst = psum.tile([P, 512], f32, tag="mm", name=f"psq{f%2}")
                for e in range(ET):
                    nc.tensor.matmul(
                        pst[:],
                        wq_all[:, f, e],
                        xt_c[j][:, e],
                        start=(e == 0),
                        stop=(e == ET - 1),
                    )
                nc.scalar.add(
                    qt_t[:, f, j * 512 : (j + 1) * 512], pst[:], bq_t[:, f : f + 1]
                )
        w_cm.__exit__(None, None, None)

        # ---- partner reconstruction: kt partner half, 2 rounds ----
        for rnd in range(2):
            if rnd == 1:
                nc.sync.dma_start(g0[:], kcc_dst[0, :, ET // 2 : ET])
                nc.sync.dma_start(g1[:], kcc_dst[1, :, ET // 2 : ET])
            for fh in range(ET // 2):
                f = rnd * (ET // 2) + fh
                for c in range(2):
                    sl = slice(c * 512, (c + 1) * 512)
                    nc.vector.tensor_add(gsum[:], g0[:, fh, sl], g1[:, fh, sl])
                    nc.vector.tensor_sub(
                        kt_t[:, 1, f, sl], gsum[:], kt_t[:, 0, f, sl]
                    )

        # V exchange: store + collective fire after the K gathers are done
        # (dummy byte write pins the queue/scheduler order)
        nc.sync.dma_start(vcc_src[0:1, 0, 0:2], g1[0:1, 0, 0:2])
        nc.sync.dma_start(vcc_src[:], v_t[:, 0:STH, :])
        nc.gpsimd.collective_compute(
            "AllGather",
            mybir.AluOpType.bypass,
            replica_groups=RG,
            ins=[vcc_src[:]],
            outs=[vcc_dst[:]],
        )

        # pt lives in the space freed by the weight + kg pools (avoids WAR
        # coupling of exp writes to unrelated late readers)
        pt_cm = tc.tile_pool(name="ptp", bufs=1)
        ptp = pt_cm.__enter__()
        pt_t = ptp.tile([P, ST, SQ], bf16, tag="pt")

        # ---- partner reconstruction: v[:, 8:16, :] (overlaps scores) ----
        vg_cm = tc.tile_pool(name="vgp", bufs=1)
        vgp = vg_cm.__enter__()
        vg = [vgp.tile([P, STH, E], bf16, tag=f"vg{i}", name=f"vg{i}") for i in range(2)]
        for i in range(2):
            nc.sync.dma_start(vg[i][:], vcc_dst[i])
        vsum_t = vgp.tile([P, 2, 512], f32, tag="vsum")
        for st in range(STH):
            for c in range(2):
                sl = slice(c * 512, (c + 1) * 512)
                nc.vector.tensor_add(vsum_t[:, c], vg[0][:, st, sl], vg[1][:, st, sl])
                nc.vector.tensor_sub(
                    v_t[:, STH + st, sl], vsum_t[:, c], v_t[:, st, sl]
                )
        vg_cm.__exit__(None, None, None)

        # ---- phase 2: scores^T + exp -> pt ----
        for k in range(ST):
            for qc in range(2):
                ps = psum.tile([P, 512], f32, tag="mm", name="pss")
                for f in range(ET):
                    nc.tensor.matmul(
                        ps[:],
                        kt_t[:, k // STH, f, (k % STH) * P : (k % STH + 1) * P],
                        qt_t[:, f, qc * 512 : (qc + 1) * 512],
                        start=(f == 0),
                        stop=(f == ET - 1),
                    )
                nc.scalar.activation(
                    pt_t[:, k, qc * 512 : (qc + 1) * 512], ps[:], ACT.Exp
                )

        # rowsums batched after scores: clean same-bank matmul streaks
        rs_cm = tc.tile_pool(name="rsp", bufs=1, space="PSUM")
        rsp = rs_cm.__enter__()
        rs_ps = [
            rsp.tile([2, 512], f32, tag=f"rsacc{qc}", name=f"rsacc{qc}")
            for qc in range(2)
        ]
        for qc in range(2):
            for k in range(ST):
                nc.tensor.matmul(
                    rs_ps[qc][:],
                    ones_t[:],
                    pt_t[:, k, qc * 512 : (qc + 1) * 512],
                    start=(k == 0),
                    stop=(k == ST - 1),
                )
        for qc in range(2):
            nc.vector.tensor_copy(rs_sb[:, qc * 512 : (qc + 1) * 512], rs_ps[qc][:])
        rs_cm.__exit__(None, None, None)

        # ---- phase 4: O = PT^T @ V, normalized ----
        prs_cm = tc.tile_pool(name="prsp", bufs=2, space="PSUM")
        prsp = prs_cm.__enter__()
        ob_cm = tc.tile_pool(name="ob", bufs=3)
        obp = ob_cm.__enter__()
        recip_t = small.tile([P, QTN], f32, tag="recip")
        for qt_i in range(QTN):
            prs = prsp.tile([P, 2], f32, tag="rs")
            nc.tensor.matmul(
                prs[:],
                rs_sb[:, qt_i * P : (qt_i + 1) * P],
                id2_t[:],
                is_transpose=True,
            )
            nc.vector.reciprocal(recip_t[:, qt_i : qt_i + 1], prs[:, 0:1])
        for qt_i in range(QTN):
            for fc in range(2):
                po = psum.tile([P, 512], f32, tag="mm", name="po")
                for k in range(ST):
                    nc.tensor.matmul(
                        po[:],
                        pt_t[:, k, qt_i * P : (qt_i + 1) * P],
                        v_t[:, k, fc * 512 : (fc + 1) * 512],
                        start=(k == 0),
                        stop=(k == ST - 1),
                    )
                o_t = obp.tile([P, 512], f32, tag="ob")
                nc.vector.tensor_scalar_mul(
                    o_t[:], po[:], recip_t[:, qt_i : qt_i + 1]
                )
                nc.scalar.dma_start(
                    o[qt_i * P : (qt_i + 1) * P, fc * 512 : (fc + 1) * 512],
                    o_t[:],
                )
        ob_cm.__exit__(None, None, None)
        prs_cm.__exit__(None, None, None)

        pt_cm.__exit__(None, None, None)
        w2 = None  # placeholder
        gat_cm.__exit__(None, None, None)
        mid_cm.__exit__(None, None, None)
        big_cm.__exit__(None, None, None)
        small_cm.__exit__(None, None, None)
        psum_cm.__exit__(None, None, None)
        dram_cm.__exit__(None, None, None)


_NC_CACHE = {}


def build_nc():
    if "nc" in _NC_CACHE:
        return _NC_CACHE["nc"]
    import concourse.bacc as bacc

    nc = bacc.Bacc(None, target_bir_lowering=False, num_devices=8)
    _emit(nc)
    nc.finalize()
    _NC_CACHE["nc"] = nc
    return nc


def make_in_maps(x, Wq, bq, Wk, bk, Wv, bv):
    """Host-side prep: per-core input dicts for run_bass_kernel_spmd."""
    import ml_dtypes

    bf16 = ml_dtypes.bfloat16
    scale = 1.0 / np.sqrt(np.float32(E))
    x = np.asarray(x, np.float32)
    B = x.shape[0]

    def tile4(wt):  # [E, E] (e rows, f cols) -> [p(e), f_tile, e_tile, c(f)]
        return np.ascontiguousarray(
            wt.reshape(ET, P, ET, P).transpose(1, 2, 0, 3)
        )

    wq4 = tile4(np.asarray(Wq, np.float32).T * scale).astype(bf16)
    wk4 = tile4(np.asarray(Wk, np.float32).T).astype(bf16)
    wkf0 = np.ascontiguousarray(wk4[:, 0])
    wvt = np.ascontiguousarray(
        np.asarray(Wv, np.float32).T.reshape(ET, P, E).transpose(1, 0, 2)
    ).astype(bf16)
    bq8 = np.ascontiguousarray((np.asarray(bq, np.float32) * scale).reshape(ET, P).T)
    bk8 = np.ascontiguousarray(np.asarray(bk, np.float32).reshape(ET, P).T)
    bvb = np.ascontiguousarray(np.broadcast_to(np.asarray(bv, np.float32), (P, E)))

    in_maps = []
    for c in range(B * 2):
        b, h = divmod(c, 2)
        xt_pm = (
            x[b].T[:, h * SQ : (h + 1) * SQ]
            .reshape(ET, P, 2, 512)
            .transpose(2, 1, 0, 3)
        )
        xt_half = np.ascontiguousarray(xt_pm).astype(bf16)
        in_maps.append(
            {
                "xt_in": xt_half,
                "wkf0_in": wkf0,
                "wq4": wq4,
                "wk4": wk4,
                "wvt_in": wvt,
                "bq8": bq8,
                "bk8": bk8,
                "bvb": bvb,
                "ones2": np.ones((P, 2), bf16),
                "id2": np.eye(2, dtype=np.float32),
            }
        )
    return in_maps


def kernel(x, Wq, bq, Wk, bk, Wv, bv):
    from concourse.bass_utils import run_bass_kernel_spmd

    x = np.asarray(x, np.float32)
    B = x.shape[0]
    nc = build_nc()
    in_maps = make_in_maps(x, Wq, bq, Wk, bk, Wv, bv)
    res = run_bass_kernel_spmd(nc, in_maps, list(range(len(in_maps))))
    out = np.empty((B, S, E), np.float32)
    for c in range(len(in_maps)):
        b, h = divmod(c, 2)
        out[b, h * SQ : (h + 1) * SQ, :] = res.results[c]["o"]
    return out


# revision 28
# speedup vs baseline: 1.1128x; 1.0706x over previous
"""Single-head attention (B=4, S=2048, E=1024, fp32) on 8 trn2 NeuronCores.

Sharding: (batch, q-half) -> 8 shards. Core c handles batch c//2, query/seq
rows [h*1024, (h+1)*1024) with h = c%2. Unlike the fp32r baseline, each core
computes K and V projections ONLY for its own 1024-row half; the partner's
half arrives via a pairwise AllGather (DRAM->DRAM). Because the program is
SPMD (identical on both cores of a pair), the partner slice index of the
gather output is rank-dependent; instead of rank addressing, the partner
half is reconstructed exactly on the vector engine as
    partner = (gather[0] + gather[1]) - own
computed in fp32 (bf16 + bf16 is exact in fp32), written back as bf16.

All matmul operands are bf16 (hw streams 2 rows/cycle vs 1 for fp32r; PSUM
accumulation is fp32). Matmul groups are emitted "A-pattern": the 8-16
matmuls of one accumulation group are consecutive into a single PSUM bank
(bank alternation costs ~60ns/matmul on hw).

Per-core k-order is own-half-first (softmax/output invariant to key order
as long as PT and V use the same order).

Device layouts (per core, partition dim first):
  xt    [P, 8(et), 1024]  x[b].T own columns       (bf16)
  qt    [P, 8(ft), 1024]  Q^T                      (bf16)
  kt    [P, 8(ft), 2048]  K^T own|partner          (bf16)
  v     [P, 16(st), 1024] V own|partner            (bf16)
  pt    [P, 16(kt), 1024] exp(scores^T)            (bf16)
"""

import numpy as np

P = 128
E = 1024
S = 2048
SQ = 1024          # q rows per core
ET = E // P        # 8 e/f tiles
QTN = SQ // P      # 8 q tiles
STH = SQ // P      # 8 k tiles per half
ST = S // P        # 16 k tiles


def _emit(nc):
    import concourse.mybir as mybir
    import concourse.tile as tile

    f32 = mybir.dt.float32
    bf16 = mybir.dt.bfloat16
    ACT = mybir.ActivationFunctionType

    xt_in = nc.dram_tensor("xt_in", [2, P, ET, 512], bf16, kind="ExternalInput")
    wkf0_in = nc.dram_tensor("wkf0_in", [P, ET, P], bf16, kind="ExternalInput")
    wq4 = nc.dram_tensor("wq4", [P, ET, ET, P], bf16, kind="ExternalInput")
    wk4 = nc.dram_tensor("wk4", [P, ET, ET, P], bf16, kind="ExternalInput")
    wvt_in = nc.dram_tensor("wvt_in", [P, ET, E], bf16, kind="ExternalInput")
    bq8 = nc.dram_tensor("bq8", [P, ET], f32, kind="ExternalInput")
    bk8 = nc.dram_tensor("bk8", [P, ET], f32, kind="ExternalInput")
    bvb = nc.dram_tensor("bvb", [P, E], f32, kind="ExternalInput")
    ones2 = nc.dram_tensor("ones2", [P, 2], bf16, kind="ExternalInput")
    id2 = nc.dram_tensor("id2", [2, 2], f32, kind="ExternalInput")
    o = nc.dram_tensor("o", [SQ, E], f32, kind="ExternalOutput")

    RG = [[0, 1], [2, 3], [4, 5], [6, 7]]

    with tile.TileContext(nc) as tc:
        dram_cm = tc.tile_pool(name="dramp", bufs=1, space="DRAM")
        dramp = dram_cm.__enter__()
        kcc_src = dramp.tile([P, ET, SQ], bf16, tag="kccs")    # [p, ft, k_own]
        kcc_dst = dramp.tile([2, P, ET, SQ], bf16, tag="kccd")
        vcc_src = dramp.tile([P, STH, E], bf16, tag="vccs")    # [p, st, f]
        vcc_dst = dramp.tile([2, P, STH, E], bf16, tag="vccd")

        psum_cm = tc.tile_pool(name="psum", bufs=4, space="PSUM")
        psum = psum_cm.__enter__()
        small_cm = tc.tile_pool(name="small", bufs=1)
        small = small_cm.__enter__()

        # long-lived big tiles
        big_cm = tc.tile_pool(name="big", bufs=1)
        big = big_cm.__enter__()
        kt_t = big.tile([P, 2, ET, SQ], bf16, tag="kt")
        v_t = big.tile([P, ST, E], bf16, tag="v")

        bq_t = small.tile([P, ET], f32, tag="bq")
        nc.gpsimd.dma_start(bq_t[:], bq8[:])
        bk_t = small.tile([P, ET], f32, tag="bk")
        nc.gpsimd.dma_start(bk_t[:], bk8[:])
        bv_t = small.tile([P, E], f32, tag="bv")
        nc.gpsimd.dma_start(bv_t[:], bvb[:])
        ones_t = small.tile([P, 2], bf16, tag="ones")
        nc.gpsimd.dma_start(ones_t[:], ones2[:])
        id2_t = small.tile([2, 2], f32, tag="id2")
        nc.gpsimd.dma_start(id2_t[:], id2[:])
        rs_sb = small.tile([2, SQ], f32, tag="rssb")

        # ---- phase 1: K (own half) -> kick CC; V (own half) -> kick CC; Q ----
        mid_cm = tc.tile_pool(name="mid", bufs=1)
        mid = mid_cm.__enter__()
        xt_c = [
            mid.tile([P, ET, 512], bf16, tag=f"xt{c}", name=f"xt{c}")
            for c in range(2)
        ]
        qt_t = mid.tile([P, ET, SQ], bf16, tag="qt")
        wvt_t = mid.tile([P, ET, E], bf16, tag="wvt")
        kg_cm = tc.tile_pool(name="kgp", bufs=1)
        kgp = kg_cm.__enter__()
        w_cm = tc.tile_pool(name="wstream", bufs=1)
        wsp = w_cm.__enter__()

        # input DMAs: partition-major DRAM layouts, single contiguous
        # transfers, split across the two HWDGE queues for parallelism
        wk_f0 = wsp.tile([P, ET, P], bf16, tag="wf0", name="wk_f0")
        nc.sync.dma_start(wk_f0[:], wkf0_in[:])
        nc.sync.dma_start(xt_c[0][:], xt_in[0])
        nc.sync.dma_start(xt_c[1][:], xt_in[1])
        wk_all = wsp.tile([P, ET, ET, P], bf16, tag="wk", name="wk_all")
        nc.scalar.dma_start(wk_all[:], wk4[:])
        wq_all = wsp.tile([P, ET, ET, P], bf16, tag="wq", name="wq_all")
        nc.sync.dma_start(wq_all[:], wq4[:])
        nc.scalar.dma_start(wvt_t[:], wvt_in[:])

        def wk_row(f):
            return wk_f0 if f == 0 else wk_all[:, f]

        # K projection (own half): out kt[:, f, 0:1024]
        kg = [
            kgp.tile([P, ET, SQ], bf16, tag=f"kg{i}", name=f"kg{i}")
            for i in range(2)
        ]
        for f in range(ET):
            for j in range(2):
                pst = psum.tile([P, 512], f32, tag="mm", name=f"psk{j}")
                for e in range(ET):
                    nc.tensor.matmul(
                        pst[:],
                        wk_row(f)[:, e],
                        xt_c[j][:, e],
                        start=(e == 0),
                        stop=(e == ET - 1),
                    )
                nc.scalar.add(
                    kt_t[:, 0, f, j * 512 : (j + 1) * 512],
                    pst[:],
                    bk_t[:, f : f + 1],
                )
        nc.sync.dma_start(kcc_src[:], kt_t[:, 0])
        nc.gpsimd.collective_compute(
            "AllGather",
            mybir.AluOpType.bypass,
            replica_groups=RG,
            ins=[kcc_src[:]],
            outs=[kcc_dst[:]],
        )
        for i in range(2):
            nc.sync.dma_start(kg[i][:], kcc_dst[i])

        # V projection (own half): out v[:, st, :] for st in 0..8
        for st in range(STH):
            for fc in range(2):
                pst = psum.tile([P, 512], f32, tag="mm", name=f"psv{fc}")
                for e in range(ET):
                    nc.tensor.matmul(
                        pst[:],
                        xt_c[st // 4][:, e, (st % 4) * P : (st % 4 + 1) * P],
                        wvt_t[:, e, fc * 512 : (fc + 1) * 512],
                        start=(e == 0),
                        stop=(e == ET - 1),
                    )
                nc.vector.tensor_add(
                    v_t[:, st, fc * 512 : (fc + 1) * 512],
                    pst[:],
                    bv_t[:, fc * 512 : (fc + 1) * 512],
                )
        # dummy byte writes order the vcc store (hence CC2) after kg loads
        nc.sync.dma_start(vcc_src[0:1, 0, 0:2], kg[0][0:1, 0, 0:2])
        nc.sync.dma_start(vcc_src[0:1, 0, 2:4], kg[1][0:1, 0, 0:2])
        nc.sync.dma_start(vcc_src[:], v_t[:, 0:STH, :])
        nc.gpsimd.collective_compute(
            "AllGather",
            mybir.AluOpType.bypass,
            replica_groups=RG,
            ins=[vcc_src[:]],
            outs=[vcc_dst[:]],
        )

        # Q projection: out qt
        for j in range(2):
            for f in range(ET):
                pst = psum.tile([P, 512], f32, tag="mm", name=f"psq{f%2}")
                for e in range(ET):
                    nc.tensor.matmul(
                        pst[:],
                        wq_all[:, f, e],
                        xt_c[j][:, e],
                        start=(e == 0),
                        stop=(e == ET - 1),
                    )
                nc.scalar.add(
                    qt_t[:, f, j * 512 : (j + 1) * 512], pst[:], bq_t[:, f : f + 1]
                )
        w_cm.__exit__(None, None, None)

        # ---- partner reconstruction: kt partner half ----
        ksum_t = kgp.tile([P, 2, 512], f32, tag="ksum")
        for f in range(ET):
            for c in range(2):
                sl = slice(c * 512, (c + 1) * 512)
                nc.vector.tensor_add(ksum_t[:, c], kg[0][:, f, sl], kg[1][:, f, sl])
                nc.vector.tensor_sub(
                    kt_t[:, 1, f, sl], ksum_t[:, c], kt_t[:, 0, f, sl]
                )

        kg_cm.__exit__(None, None, None)

        # pt lives in the space freed by the weight + kg pools (avoids WAR
        # coupling of exp writes to unrelated late readers)
        pt_cm = tc.tile_pool(name="ptp", bufs=1)
        ptp = pt_cm.__enter__()
        pt_t = ptp.tile([P, ST, SQ], bf16, tag="pt")

        # ---- partner reconstruction: v[:, 8:16, :] (overlaps scores) ----
        vg_cm = tc.tile_pool(name="vgp", bufs=1)
        vgp = vg_cm.__enter__()
        vg = [vgp.tile([P, STH, E], bf16, tag=f"vg{i}", name=f"vg{i}") for i in range(2)]
        for i in range(2):
            nc.sync.dma_start(vg[i][:], vcc_dst[i])
        vsum_t = vgp.tile([P, 2, 512], f32, tag="vsum")
        for st in range(STH):
            for c in range(2):
                sl = slice(c * 512, (c + 1) * 512)
                nc.vector.tensor_add(vsum_t[:, c], vg[0][:, st, sl], vg[1][:, st, sl])
                nc.vector.tensor_sub(
                    v_t[:, STH + st, sl], vsum_t[:, c], v_t[:, st, sl]
                )
        vg_cm.__exit__(None, None, None)

        # ---- phase 2: scores^T + exp -> pt ----
        for k in range(ST):
            for qc in range(2):
                ps = psum.tile([P, 512], f32, tag="mm", name="pss")
                for f in range(ET):
                    nc.tensor.matmul(
                        ps[:],
                        kt_t[:, k // STH, f, (k % STH) * P : (k % STH + 1) * P],
                        qt_t[:, f, qc * 512 : (qc + 1) * 512],
                        start=(f == 0),
                        stop=(f == ET - 1),
                    )
                nc.scalar.activation(
                    pt_t[:, k, qc * 512 : (qc + 1) * 512], ps[:], ACT.Exp
                )

        # rowsums batched after scores: clean same-bank matmul streaks
        rs_cm = tc.tile_pool(name="rsp", bufs=1, space="PSUM")
        rsp = rs_cm.__enter__()
        rs_ps = [
            rsp.tile([2, 512], f32, tag=f"rsacc{qc}", name=f"rsacc{qc}")
            for qc in range(2)
        ]
        for qc in range(2):
            for k in range(ST):
                nc.tensor.matmul(
                    rs_ps[qc][:],
                    ones_t[:],
                    pt_t[:, k, qc * 512 : (qc + 1) * 512],
                    start=(k == 0),
                    stop=(k == ST - 1),
                )
        for qc in range(2):
            nc.vector.tensor_copy(rs_sb[:, qc * 512 : (qc + 1) * 512], rs_ps[qc][:])
        rs_cm.__exit__(None, None, None)

        # ---- phase 4: O = PT^T @ V, normalized ----
        prs_cm = tc.tile_pool(name="prsp", bufs=2, space="PSUM")
        prsp = prs_cm.__enter__()
        ob_cm = tc.tile_pool(name="ob", bufs=3)
        obp = ob_cm.__enter__()
        recip_t = small.tile([P, QTN], f32, tag="recip")
        for qt_i in range(QTN):
            prs = prsp.tile([P, 2], f32, tag="rs")
            nc.tensor.matmul(
                prs[:],
                rs_sb[:, qt_i * P : (qt_i + 1) * P],
                id2_t[:],
                is_transpose=True,
            )
            nc.vector.reciprocal(recip_t[:, qt_i : qt_i + 1], prs[:, 0:1])
        for qt_i in range(QTN):
            for fc in range(2):
                po = psum.tile([P, 512], f32, tag="mm", name="po")
                for k in range(ST):
                    nc.tensor.matmul(
                        po[:],
                        pt_t[:, k, qt_i * P : (qt_i + 1) * P],
                        v_t[:, k, fc * 512 : (fc + 1) * 512],
                        start=(k == 0),
                        stop=(k == ST - 1),
                    )
                o_t = obp.tile([P, 512], f32, tag="ob")
                nc.vector.tensor_scalar_mul(
                    o_t[:], po[:], recip_t[:, qt_i : qt_i + 1]
                )
                nc.scalar.dma_start(
                    o[qt_i * P : (qt_i + 1) * P, fc * 512 : (fc + 1) * 512],
                    o_t[:],
                )
        ob_cm.__exit__(None, None, None)
        prs_cm.__exit__(None, None, None)

        pt_cm.__exit__(None, None, None)
        mid_cm.__exit__(None, None, None)
        big_cm.__exit__(None, None, None)
        small_cm.__exit__(None, None, None)
        psum_cm.__exit__(None, None, None)
        dram_cm.__exit__(None, None, None)


_NC_CACHE = {}


def build_nc():
    if "nc" in _NC_CACHE:
        return _NC_CACHE["nc"]
    import concourse.bacc as bacc

    nc = bacc.Bacc(None, target_bir_lowering=False, num_devices=8)
    _emit(nc)
    nc.finalize()
    _NC_CACHE["nc"] = nc
    return nc


def make_in_maps(x, Wq, bq, Wk, bk, Wv, bv):
    """Host-side prep: per-core input dicts for run_bass_kernel_spmd."""
    import ml_dtypes

    bf16 = ml_dtypes.bfloat16
    scale = 1.0 / np.sqrt(np.float32(E))
    x = np.asarray(x, np.float32)
    B = x.shape[0]

    def tile4(wt):  # [E, E] (e rows, f cols) -> [p(e), f_tile, e_tile, c(f)]
        return np.ascontiguousarray(
            wt.reshape(ET, P, ET, P).transpose(1, 2, 0, 3)
        )

    wq4 = tile4(np.asarray(Wq, np.float32).T * scale).astype(bf16)
    wk4 = tile4(np.asarray(Wk, np.float32).T).astype(bf16)
    wkf0 = np.ascontiguousarray(wk4[:, 0])
    wvt = np.ascontiguousarray(
        np.asarray(Wv, np.float32).T.reshape(ET, P, E).transpose(1, 0, 2)
    ).astype(bf16)
    bq8 = np.ascontiguousarray((np.asarray(bq, np.float32) * scale).reshape(ET, P).T)
    bk8 = np.ascontiguousarray(np.asarray(bk, np.float32).reshape(ET, P).T)
    bvb = np.ascontiguousarray(np.broadcast_to(np.asarray(bv, np.float32), (P, E)))

    in_maps = []
    for c in range(B * 2):
        b, h = divmod(c, 2)
        xt_pm = (
            x[b].T[:, h * SQ : (h + 1) * SQ]
            .reshape(ET, P, 2, 512)
            .transpose(2, 1, 0, 3)
        )
        xt_half = np.ascontiguousarray(xt_pm).astype(bf16)
        in_maps.append(
            {
                "xt_in": xt_half,
                "wkf0_in": wkf0,
                "wq4": wq4,
                "wk4": wk4,
                "wvt_in": wvt,
                "bq8": bq8,
                "bk8": bk8,
                "bvb": bvb,
                "ones2": np.ones((P, 2), bf16),
                "id2": np.eye(2, dtype=np.float32),
            }
        )
    return in_maps


def kernel(x, Wq, bq, Wk, bk, Wv, bv):
    from concourse.bass_utils import run_bass_kernel_spmd

    x = np.asarray(x, np.float32)
    B = x.shape[0]
    nc = build_nc()
    in_maps = make_in_maps(x, Wq, bq, Wk, bk, Wv, bv)
    res = run_bass_kernel_spmd(nc, in_maps, list(range(len(in_maps))))
    out = np.empty((B, S, E), np.float32)
    for c in range(len(in_maps)):
        b, h = divmod(c, 2)
        out[b, h * SQ : (h + 1) * SQ, :] = res.results[c]["o"]
    return out


# revision 29
# speedup vs baseline: 1.1879x; 1.0675x over previous
"""Single-head attention (B=4, S=2048, E=1024, fp32) on 8 trn2 NeuronCores.

Sharding: (batch, q-half) -> 8 shards. Core c handles batch c//2, query/seq
rows [h*1024, (h+1)*1024) with h = c%2. Unlike the fp32r baseline, each core
computes K and V projections ONLY for its own 1024-row half; the partner's
half arrives via a pairwise AllGather (DRAM->DRAM). Because the program is
SPMD (identical on both cores of a pair), the partner slice index of the
gather output is rank-dependent; instead of rank addressing, the partner
half is reconstructed exactly on the vector engine as
    partner = (gather[0] + gather[1]) - own
computed in fp32 (bf16 + bf16 is exact in fp32), written back as bf16.

All matmul operands are bf16 (same ~215-260ns per 512-wide matmul cadence
as fp32r on this hw, but half the SBUF/DMA footprint, which is what lets
K^T, V, and exp(scores^T) stay SBUF-resident with no DRAM bounce). PSUM
accumulation is fp32. Matmul groups keep the 8-16 matmuls of an
accumulation group consecutive into a single PSUM bank (bank alternation
costs ~60ns/matmul). Input DMAs use partition-major DRAM layouts so every
transfer is a single contiguous descriptor per partition.

Per-core k-order is own-half-first (softmax/output invariant to key order
as long as PT and V use the same order).

Device layouts (per core, partition dim first):
  xt    [P, 8(et), 1024]  x[b].T own columns       (bf16)
  qt    [P, 8(ft), 1024]  Q^T                      (bf16)
  kt    [P, 8(ft), 2048]  K^T own|partner          (bf16)
  v     [P, 16(st), 1024] V own|partner            (bf16)
  pt    [P, 16(kt), 1024] exp(scores^T)            (bf16)
"""

import numpy as np

P = 128
E = 1024
S = 2048
SQ = 1024          # q rows per core
ET = E // P        # 8 e/f tiles
QTN = SQ // P      # 8 q tiles
STH = SQ // P      # 8 k tiles per half
ST = S // P        # 16 k tiles


def _emit(nc):
    import concourse.mybir as mybir
    import concourse.tile as tile

    f32 = mybir.dt.float32
    bf16 = mybir.dt.bfloat16
    ACT = mybir.ActivationFunctionType

    xt_in = nc.dram_tensor("xt_in", [2, P, ET, 512], bf16, kind="ExternalInput")
    wkf0_in = nc.dram_tensor("wkf0_in", [P, ET, P], bf16, kind="ExternalInput")
    wq4 = nc.dram_tensor("wq4", [P, ET, ET, P], bf16, kind="ExternalInput")
    wk4 = nc.dram_tensor("wk4", [P, ET, ET, P], bf16, kind="ExternalInput")
    wvt_in = nc.dram_tensor("wvt_in", [P, ET, E], bf16, kind="ExternalInput")
    bq8 = nc.dram_tensor("bq8", [P, ET], f32, kind="ExternalInput")
    bk8 = nc.dram_tensor("bk8", [P, ET], f32, kind="ExternalInput")
    bvb = nc.dram_tensor("bvb", [P, E], f32, kind="ExternalInput")
    ones2 = nc.dram_tensor("ones2", [P, 2], bf16, kind="ExternalInput")
    id2 = nc.dram_tensor("id2", [2, 2], f32, kind="ExternalInput")
    o = nc.dram_tensor("o", [SQ, E], f32, kind="ExternalOutput")

    RG = [[0, 1], [2, 3], [4, 5], [6, 7]]

    with tile.TileContext(nc) as tc:
        dram_cm = tc.tile_pool(name="dramp", bufs=1, space="DRAM")
        dramp = dram_cm.__enter__()
        kcc_src = dramp.tile([P, ET, SQ], bf16, tag="kccs")    # [p, ft, k_own]
        kcc_dst = dramp.tile([2, P, ET, SQ], bf16, tag="kccd")
        vcc_src = dramp.tile([P, STH, E], bf16, tag="vccs")    # [p, st, f]
        vcc_dst = dramp.tile([2, P, STH, E], bf16, tag="vccd")

        psum_cm = tc.tile_pool(name="psum", bufs=4, space="PSUM")
        psum = psum_cm.__enter__()
        small_cm = tc.tile_pool(name="small", bufs=1)
        small = small_cm.__enter__()

        # long-lived big tiles
        big_cm = tc.tile_pool(name="big", bufs=1)
        big = big_cm.__enter__()
        kt_t = big.tile([P, 2, ET, SQ], bf16, tag="kt")
        v_t = big.tile([P, ST, E], bf16, tag="v")

        bq_t = small.tile([P, ET], f32, tag="bq")
        nc.gpsimd.dma_start(bq_t[:], bq8[:])
        bk_t = small.tile([P, ET], f32, tag="bk")
        nc.gpsimd.dma_start(bk_t[:], bk8[:])
        bv_t = small.tile([P, E], f32, tag="bv")
        nc.gpsimd.dma_start(bv_t[:], bvb[:])
        ones_t = small.tile([P, 2], bf16, tag="ones")
        nc.gpsimd.dma_start(ones_t[:], ones2[:])
        id2_t = small.tile([2, 2], f32, tag="id2")
        nc.gpsimd.dma_start(id2_t[:], id2[:])
        rs_sb = small.tile([2, SQ], f32, tag="rssb")

        # ---- phase 1: K (own half) -> kick CC; V (own half) -> kick CC; Q ----
        mid_cm = tc.tile_pool(name="mid", bufs=1)
        mid = mid_cm.__enter__()
        xt_c = [
            mid.tile([P, ET, 512], bf16, tag=f"xt{c}", name=f"xt{c}")
            for c in range(2)
        ]
        qt_t = mid.tile([P, ET, SQ], bf16, tag="qt")
        wvt_t = mid.tile([P, ET, E], bf16, tag="wvt")
        kg_cm = tc.tile_pool(name="kgp", bufs=1)
        kgp = kg_cm.__enter__()
        w_cm = tc.tile_pool(name="wstream", bufs=1)
        wsp = w_cm.__enter__()

        # input DMAs: partition-major DRAM layouts, single contiguous
        # transfers, split across the two HWDGE queues for parallelism
        wk_f0 = wsp.tile([P, ET, P], bf16, tag="wf0", name="wk_f0")
        nc.sync.dma_start(wk_f0[:], wkf0_in[:])
        nc.sync.dma_start(xt_c[0][:], xt_in[0])
        nc.sync.dma_start(xt_c[1][:], xt_in[1])
        wk_all = wsp.tile([P, ET, ET, P], bf16, tag="wk", name="wk_all")
        nc.scalar.dma_start(wk_all[:], wk4[:])
        wq_all = wsp.tile([P, ET, ET, P], bf16, tag="wq", name="wq_all")
        nc.sync.dma_start(wq_all[:], wq4[:])
        nc.scalar.dma_start(wvt_t[:], wvt_in[:])

        def wk_row(f):
            return wk_f0 if f == 0 else wk_all[:, f]

        # K projection (own half): out kt[:, f, 0:1024]
        kg = [
            kgp.tile([P, ET, SQ], bf16, tag=f"kg{i}", name=f"kg{i}")
            for i in range(2)
        ]
        for f in range(ET):
            for j in range(2):
                pst = psum.tile([P, 512], f32, tag="mm", name=f"psk{j}")
                for e in range(ET):
                    nc.tensor.matmul(
                        pst[:],
                        wk_row(f)[:, e],
                        xt_c[j][:, e],
                        start=(e == 0),
                        stop=(e == ET - 1),
                    )
                nc.scalar.add(
                    kt_t[:, 0, f, j * 512 : (j + 1) * 512],
                    pst[:],
                    bk_t[:, f : f + 1],
                )
        nc.sync.dma_start(kcc_src[:], kt_t[:, 0])
        nc.gpsimd.collective_compute(
            "AllGather",
            mybir.AluOpType.bypass,
            replica_groups=RG,
            ins=[kcc_src[:]],
            outs=[kcc_dst[:]],
        )
        for i in range(2):
            nc.sync.dma_start(kg[i][:], kcc_dst[i])

        # V projection (own half): out v[:, st, :] for st in 0..8
        for st in range(STH):
            for fc in range(2):
                pst = psum.tile([P, 512], f32, tag="mm", name=f"psv{fc}")
                for e in range(ET):
                    nc.tensor.matmul(
                        pst[:],
                        xt_c[st // 4][:, e, (st % 4) * P : (st % 4 + 1) * P],
                        wvt_t[:, e, fc * 512 : (fc + 1) * 512],
                        start=(e == 0),
                        stop=(e == ET - 1),
                    )
                nc.vector.tensor_add(
                    v_t[:, st, fc * 512 : (fc + 1) * 512],
                    pst[:],
                    bv_t[:, fc * 512 : (fc + 1) * 512],
                )
        # dummy byte writes order the vcc store (hence CC2) after kg loads
        nc.sync.dma_start(vcc_src[0:1, 0, 0:2], kg[0][0:1, 0, 0:2])
        nc.sync.dma_start(vcc_src[0:1, 0, 2:4], kg[1][0:1, 0, 0:2])
        nc.sync.dma_start(vcc_src[:], v_t[:, 0:STH, :])
        nc.gpsimd.collective_compute(
            "AllGather",
            mybir.AluOpType.bypass,
            replica_groups=RG,
            ins=[vcc_src[:]],
            outs=[vcc_dst[:]],
        )

        # Q projection: out qt
        for j in range(2):
            for f in range(ET):
                pst = psum.tile([P, 512], f32, tag="mm", name=f"psq{f%2}")
                for e in range(ET):
                    nc.tensor.matmul(
                        pst[:],
                        wq_all[:, f, e],
                        xt_c[j][:, e],
                        start=(e == 0),
                        stop=(e == ET - 1),
                    )
                nc.scalar.add(
                    qt_t[:, f, j * 512 : (j + 1) * 512], pst[:], bq_t[:, f : f + 1]
                )
        w_cm.__exit__(None, None, None)

        # ---- partner reconstruction: kt partner half ----
        ksum_t = kgp.tile([P, 2, 512], f32, tag="ksum")
        for f in range(ET):
            for c in range(2):
                sl = slice(c * 512, (c + 1) * 512)
                nc.vector.tensor_add(ksum_t[:, c], kg[0][:, f, sl], kg[1][:, f, sl])
                nc.vector.tensor_sub(
                    kt_t[:, 1, f, sl], ksum_t[:, c], kt_t[:, 0, f, sl]
                )

        kg_cm.__exit__(None, None, None)

        # pt lives in the space freed by the weight + kg pools (avoids WAR
        # coupling of exp writes to unrelated late readers)
        pt_cm = tc.tile_pool(name="ptp", bufs=1)
        ptp = pt_cm.__enter__()
        pt_t = ptp.tile([P, ST, SQ], bf16, tag="pt")

        # ---- partner reconstruction: v[:, 8:16, :] (overlaps scores) ----
        vg_cm = tc.tile_pool(name="vgp", bufs=1)
        vgp = vg_cm.__enter__()
        vg = [vgp.tile([P, STH, E], bf16, tag=f"vg{i}", name=f"vg{i}") for i in range(2)]
        for i in range(2):
            nc.sync.dma_start(vg[i][:], vcc_dst[i])
        vsum_t = vgp.tile([P, 2, 512], f32, tag="vsum")
        for st in range(STH):
            for c in range(2):
                sl = slice(c * 512, (c + 1) * 512)
                nc.vector.tensor_add(vsum_t[:, c], vg[0][:, st, sl], vg[1][:, st, sl])
                nc.vector.tensor_sub(
                    v_t[:, STH + st, sl], vsum_t[:, c], v_t[:, st, sl]
                )
        vg_cm.__exit__(None, None, None)

        # ---- phase 2: scores^T + exp -> pt ----
        for k in range(ST):
            for qc in range(2):
                ps = psum.tile([P, 512], f32, tag="mm", name="pss")
                for f in range(ET):
                    nc.tensor.matmul(
                        ps[:],
                        kt_t[:, k // STH, f, (k % STH) * P : (k % STH + 1) * P],
                        qt_t[:, f, qc * 512 : (qc + 1) * 512],
                        start=(f == 0),
                        stop=(f == ET - 1),
                    )
                nc.scalar.activation(
                    pt_t[:, k, qc * 512 : (qc + 1) * 512], ps[:], ACT.Exp
                )

        # rowsums batched after scores: clean same-bank matmul streaks
        rs_cm = tc.tile_pool(name="rsp", bufs=1, space="PSUM")
        rsp = rs_cm.__enter__()
        rs_ps = [
            rsp.tile([2, 512], f32, tag=f"rsacc{qc}", name=f"rsacc{qc}")
            for qc in range(2)
        ]
        for qc in range(2):
            for k in range(ST):
                nc.tensor.matmul(
                    rs_ps[qc][:],
                    ones_t[:],
                    pt_t[:, k, qc * 512 : (qc + 1) * 512],
                    start=(k == 0),
                    stop=(k == ST - 1),
                )
        for qc in range(2):
            nc.vector.tensor_copy(rs_sb[:, qc * 512 : (qc + 1) * 512], rs_ps[qc][:])
        rs_cm.__exit__(None, None, None)

        # ---- phase 4: O = PT^T @ V, normalized ----
        prs_cm = tc.tile_pool(name="prsp", bufs=2, space="PSUM")
        prsp = prs_cm.__enter__()
        ob_cm = tc.tile_pool(name="ob", bufs=3)
        obp = ob_cm.__enter__()
        recip_t = small.tile([P, QTN], f32, tag="recip")
        for qt_i in range(QTN):
            prs = prsp.tile([P, 2], f32, tag="rs")
            nc.tensor.matmul(
                prs[:],
                rs_sb[:, qt_i * P : (qt_i + 1) * P],
                id2_t[:],
                is_transpose=True,
            )
            nc.vector.reciprocal(recip_t[:, qt_i : qt_i + 1], prs[:, 0:1])
        for qt_i in range(QTN):
            for fc in range(2):
                po = psum.tile([P, 512], f32, tag="mm", name="po")
                for k in range(ST):
                    nc.tensor.matmul(
                        po[:],
                        pt_t[:, k, qt_i * P : (qt_i + 1) * P],
                        v_t[:, k, fc * 512 : (fc + 1) * 512],
                        start=(k == 0),
                        stop=(k == ST - 1),
                    )
                o_t = obp.tile([P, 512], f32, tag="ob")
                nc.vector.tensor_scalar_mul(
                    o_t[:], po[:], recip_t[:, qt_i : qt_i + 1]
                )
                nc.scalar.dma_start(
                    o[qt_i * P : (qt_i + 1) * P, fc * 512 : (fc + 1) * 512],
                    o_t[:],
                )
        ob_cm.__exit__(None, None, None)
        prs_cm.__exit__(None, None, None)

        pt_cm.__exit__(None, None, None)
        mid_cm.__exit__(None, None, None)
        big_cm.__exit__(None, None, None)
        small_cm.__exit__(None, None, None)
        psum_cm.__exit__(None, None, None)
        dram_cm.__exit__(None, None, None)


_NC_CACHE = {}


def build_nc():
    if "nc" in _NC_CACHE:
        return _NC_CACHE["nc"]
    import concourse.bacc as bacc

    nc = bacc.Bacc(None, target_bir_lowering=False, num_devices=8)
    _emit(nc)
    nc.finalize()
    _NC_CACHE["nc"] = nc
    return nc


def make_in_maps(x, Wq, bq, Wk, bk, Wv, bv):
    """Host-side prep: per-core input dicts for run_bass_kernel_spmd."""
    import ml_dtypes

    bf16 = ml_dtypes.bfloat16
    scale = 1.0 / np.sqrt(np.float32(E))
    x = np.asarray(x, np.float32)
    B = x.shape[0]

    def tile4(wt):  # [E, E] (e rows, f cols) -> [p(e), f_tile, e_tile, c(f)]
        return np.ascontiguousarray(
            wt.reshape(ET, P, ET, P).transpose(1, 2, 0, 3)
        )

    wq4 = tile4(np.asarray(Wq, np.float32).T * scale).astype(bf16)
    wk4 = tile4(np.asarray(Wk, np.float32).T).astype(bf16)
    wkf0 = np.ascontiguousarray(wk4[:, 0])
    wvt = np.ascontiguousarray(
        np.asarray(Wv, np.float32).T.reshape(ET, P, E).transpose(1, 0, 2)
    ).astype(bf16)
    bq8 = np.ascontiguousarray((np.asarray(bq, np.float32) * scale).reshape(ET, P).T)
    bk8 = np.ascontiguousarray(np.asarray(bk, np.float32).reshape(ET, P).T)
    bvb = np.ascontiguousarray(np.broadcast_to(np.asarray(bv, np.float32), (P, E)))

    in_maps = []
    for c in range(B * 2):
        b, h = divmod(c, 2)
        xt_pm = (
            x[b].T[:, h * SQ : (h + 1) * SQ]
            .reshape(ET, P, 2, 512)
            .transpose(2, 1, 0, 3)
        )
        xt_half = np.ascontiguousarray(xt_pm).astype(bf16)
        in_maps.append(
            {
                "xt_in": xt_half,
                "wkf0_in": wkf0,
                "wq4": wq4,
                "wk4": wk4,
                "wvt_in": wvt,
                "bq8": bq8,
                "bk8": bk8,
                "bvb": bvb,
                "ones2": np.ones((P, 2), bf16),
                "id2": np.eye(2, dtype=np.float32),
            }
        )
    return in_maps


def kernel(x, Wq, bq, Wk, bk, Wv, bv):
    from concourse.bass_utils import run_bass_kernel_spmd

    x = np.asarray(x, np.float32)
    B = x.shape[0]
    nc = build_nc()
    in_maps = make_in_maps(x, Wq, bq, Wk, bk, Wv, bv)
    res = run_bass_kernel_spmd(nc, in_maps, list(range(len(in_maps))))
    out = np.empty((B, S, E), np.float32)
    for c in range(len(in_maps)):
        b, h = divmod(c, 2)
        out[b, h * SQ : (h + 1) * SQ, :] = res.results[c]["o"]
    return out
